# revision 1
# baseline (speedup 1.0000x reference)
"""Self-contained Trainium2 Bass kernel for a 3-layer GCN (EnhancedGCN).

Strategy (8 NeuronCores):
  - Nodes are partitioned contiguously: core c owns nodes [c*12500, (c+1)*12500),
    padded to 12544 = 98*128 slots per core (padded global id space of 100352).
  - Edges are assigned by destination core. Per destination-tile (128 nodes) and
    per source-window (quarter of the padded id space, 25088 rows -> int16
    indexable), edges are grouped, padded to CAP with null entries.
  - Per layer: dense matmul of the own shard (h @ W) -> AllGather of the
    transformed feature table -> per-edge feature rows fetched with the
    dma_gather custom DMA instruction -> scatter into 128-node output tiles via
    TensorE matmuls against one-hot "scatter matrices" built on DVE from
    destination offsets and edge norms (aggregation kept transposed
    [feat x node] so the per-feature bias rides the ScalarE activation).
  - Self-loops are one extra matmul per tile reading the own shard directly
    (no gather, no dependency on the AllGather).
"""

import sys

sys.path.insert(0, "/opt/trn_rl_repo")

import numpy as np

# ---------------------------------------------------------------- problem cfg
N_CORES = 8


def make_cfg(n_nodes, own, d_in, d_hid, d_out, cap):
    ntiles = (own + 127) // 128
    shard = ntiles * 128
    padn = N_CORES * shard
    assert padn % 4 == 0
    window = padn // 4
    assert window <= 32768
    assert cap % 128 == 0
    bpg = cap // 128  # gather blocks per (tile, window) group
    blk = 4 * bpg + 1  # metadata blocks per tile (incl. self block)
    return dict(
        n_nodes=n_nodes, own=own, ntiles=ntiles, shard=shard, padn=padn,
        window=window, cap=cap, bpg=bpg, blk=blk,
        d_in=d_in, d_hid=d_hid, d_out=d_out,
    )


FULL_CFG = make_cfg(n_nodes=100000, own=12500, d_in=128, d_hid=128, d_out=64, cap=640)


# ---------------------------------------------------------------- host prep
def prep_meta(cfg, edge_index):
    """Build per-core gather/scatter metadata from the edge list.

    Returns list (per core) of dicts with gidx/gdo/gnm arrays, plus norms.
    """
    n = cfg["n_nodes"]
    own, ntiles, shard = cfg["own"], cfg["ntiles"], cfg["shard"]
    window, cap, bpg, blk = cfg["window"], cfg["cap"], cfg["bpg"], cfg["blk"]

    src = np.asarray(edge_index[0], dtype=np.int64)
    dst = np.asarray(edge_index[1], dtype=np.int64)
    deg = np.bincount(dst, minlength=n).astype(np.float64) + 1.0  # + self loop
    dinv = 1.0 / np.sqrt(deg)
    norm_e = (dinv[src] * dinv[dst]).astype(np.float32)
    norm_self = (dinv * dinv).astype(np.float32)

    spid = (src // own) * shard + (src % own)  # padded global source id
    w_of = spid // window
    lidx = spid % window
    core_of = dst // own

    per_core = []
    for c in range(N_CORES):
        sel = core_of == c
        s_l, d_l = lidx[sel], dst[sel]
        s_w = w_of[sel]
        nm = norm_e[sel]
        slot = d_l % own
        t = slot // 128
        off = slot % 128

        order = np.lexsort((s_l, s_w, t))
        s_l, s_w, nm, t, off = s_l[order], s_w[order], nm[order], t[order], off[order]

        gidx = np.zeros((ntiles * 4, cap), dtype=np.int16)
        gdo = np.full((ntiles, blk, 128), -1.0, dtype=np.float32)
        gnm = np.zeros((ntiles, blk, 128), dtype=np.float32)

        # group boundaries
        gkey = t * 4 + s_w
        counts = np.bincount(gkey, minlength=ntiles * 4)
        if counts.max() > cap:
            raise ValueError(f"group overflow: {counts.max()} > cap {cap}")
        starts = np.concatenate([[0], np.cumsum(counts)])
        for g in range(ntiles * 4):
            a, b = starts[g], starts[g + 1]
            cnt = b - a
            if cnt == 0:
                continue
            gt, gw = g // 4, g % 4
            gidx[g, :cnt] = s_l[a:b].astype(np.int16)
            base = gw * bpg
            flat_do = gdo[gt].reshape(blk * 128)
            flat_nm = gnm[gt].reshape(blk * 128)
            flat_do[base * 128 : base * 128 + cnt] = off[a:b].astype(np.float32)
            flat_nm[base * 128 : base * 128 + cnt] = nm[a:b]

        # self block (index blk-1): diag norm for own nodes of this tile
        for gt in range(ntiles):
            node0 = c * own  # original id base for this core
            p = np.arange(128)
            slots = gt * 128 + p
            valid = slots < own
            gdo[gt, blk - 1, :] = p.astype(np.float32)
            gnm[gt, blk - 1, valid] = norm_self[node0 + slots[valid]]

        # wrap gidx into the 16-partition layout, replicated to 128 partitions
        # entry (g, i) -> partition i % 16, col g*(cap//16) + i//16
        gidx_w = gidx.reshape(ntiles * 4, cap // 16, 16).transpose(2, 0, 1)
        gidx_w = gidx_w.reshape(16, ntiles * 4 * (cap // 16))
        gidx_w = np.tile(gidx_w, (8, 1))  # [128, ...]

        # gdo/gnm: [ntiles, blk, 128] -> [128, ntiles*blk]
        gdo_w = gdo.reshape(ntiles * blk, 128).T.copy()
        gnm_w = gnm.reshape(ntiles * blk, 128).T.copy()

        per_core.append(dict(gidx=gidx_w, gdo=gdo_w, gnm=gnm_w))
    return per_core


def prep_inputs(cfg, x, edge_index, W1, b1, W2, b2, W3, b3):
    own, shard = cfg["own"], cfg["shard"]
    d_in, d_hid, d_out = cfg["d_in"], cfg["d_hid"], cfg["d_out"]
    meta = prep_meta(cfg, edge_index)
    x = np.asarray(x, dtype=np.float32)
    iota = np.tile(np.arange(128, dtype=np.float32), (128, 1))
    in_maps = []
    for c in range(N_CORES):
        xs = x[c * own : (c + 1) * own]  # [own, d_in]
        xT = np.zeros((d_in, shard), dtype=np.float32)
        xT[:, :own] = xs.T
        in_maps.append(
            {
                "xT": xT,
                "W1": np.asarray(W1, dtype=np.float32),
                "b1": np.asarray(b1, dtype=np.float32).reshape(d_hid, 1),
                "W2": np.asarray(W2, dtype=np.float32),
                "b2": np.asarray(b2, dtype=np.float32).reshape(d_hid, 1),
                "W3": np.asarray(W3, dtype=np.float32),
                "b3": np.asarray(b3, dtype=np.float32).reshape(d_out, 1),
                "I128": iota,
                "gidx": meta[c]["gidx"],
                "gdo": meta[c]["gdo"],
                "gnm": meta[c]["gnm"],
            }
        )
    return in_maps


# ---------------------------------------------------------------- bass program
def build_program(cfg):
    from concourse import bacc, tile, mybir

    f32 = mybir.dt.float32
    i16 = mybir.dt.int16
    ntiles, shard, padn = cfg["ntiles"], cfg["shard"], cfg["padn"]
    window, cap, bpg, blk = cfg["window"], cfg["cap"], cfg["bpg"], cfg["blk"]
    d_in, d_hid, d_out = cfg["d_in"], cfg["d_hid"], cfg["d_out"]
    icols = cap // 16

    nc = bacc.Bacc("TRN2", target_bir_lowering=False, debug=False,
                   num_devices=N_CORES)

    xT_d = nc.dram_tensor("xT", [d_in, shard], f32, kind="ExternalInput")
    W1_d = nc.dram_tensor("W1", [d_in, d_hid], f32, kind="ExternalInput")
    b1_d = nc.dram_tensor("b1", [d_hid, 1], f32, kind="ExternalInput")
    W2_d = nc.dram_tensor("W2", [d_hid, d_hid], f32, kind="ExternalInput")
    b2_d = nc.dram_tensor("b2", [d_hid, 1], f32, kind="ExternalInput")
    W3_d = nc.dram_tensor("W3", [d_hid, d_out], f32, kind="ExternalInput")
    b3_d = nc.dram_tensor("b3", [d_out, 1], f32, kind="ExternalInput")
    I_d = nc.dram_tensor("I128", [128, 128], f32, kind="ExternalInput")
    gidx_d = nc.dram_tensor("gidx", [128, ntiles * 4 * icols], i16, kind="ExternalInput")
    gdo_d = nc.dram_tensor("gdo", [128, ntiles * blk], f32, kind="ExternalInput")
    gnm_d = nc.dram_tensor("gnm", [128, ntiles * blk], f32, kind="ExternalInput")
    out_d = nc.dram_tensor("out", [d_out, shard], f32, kind="ExternalOutput")

    rg = [list(range(N_CORES))]

    with tile.TileContext(nc) as tc:
        with (
            tc.tile_pool(name="const", bufs=1) as constp,
            tc.tile_pool(name="acc", bufs=1) as accp,
            tc.tile_pool(name="lt", bufs=3) as ltp,
            tc.tile_pool(name="hw", bufs=3) as hwp,
            tc.tile_pool(name="meta", bufs=3) as metap,
            tc.tile_pool(name="gth", bufs=3) as gthp,
            tc.tile_pool(name="epool", bufs=4) as ep,
            tc.tile_pool(name="outp", bufs=3) as outp,
            tc.tile_pool(name="dpsum", bufs=2, space="PSUM") as dpsum,
            tc.tile_pool(name="apsum", bufs=2, space="PSUM") as apsum,
            tc.tile_pool(name="dram", bufs=1, space="DRAM") as dram,
        ):
            # constants
            I128 = constp.tile([128, 128], f32, name="I128c")
            nc.sync.dma_start(I128[:], I_d[:])
            W1 = constp.tile([d_in, d_hid], f32, name="W1c")
            nc.sync.dma_start(W1[:], W1_d[:])
            W2 = constp.tile([d_hid, d_hid], f32, name="W2c")
            nc.sync.dma_start(W2[:], W2_d[:])
            W3 = constp.tile([d_hid, d_out], f32, name="W3c")
            nc.sync.dma_start(W3[:], W3_d[:])
            b1 = constp.tile([d_hid, 1], f32, name="b1c")
            nc.sync.dma_start(b1[:], b1_d[:])
            b2 = constp.tile([d_hid, 1], f32, name="b2c")
            nc.sync.dma_start(b2[:], b2_d[:])
            b3 = constp.tile([d_out, 1], f32, name="b3c")
            nc.sync.dma_start(b3[:], b3_d[:])

            acc = accp.tile([128, shard], f32, name="accbuf")

            layers = [
                dict(l=1, din=d_in, dout=d_hid, W=W1, b=b1, relu=True),
                dict(l=2, din=d_hid, dout=d_hid, W=W2, b=b2, relu=True),
                dict(l=3, din=d_hid, dout=d_out, W=W3, b=b3, relu=False),
            ]

            for L in layers:
                l, din, dout, W, b = L["l"], L["din"], L["dout"], L["W"], L["b"]
                agin = dram.tile([shard, dout], f32, name=f"agin{l}")
                table = dram.tile([padn, dout], f32, addr_space="Shared",
                                  name=f"table{l}")

                # ---- dense: hw = own_shard @ W  (row-major into agin)
                for nt in range(ntiles):
                    if l == 1:
                        lt = ltp.tile([d_in, 128], f32, name="lt")
                        nc.sync.dma_start(lt[:], xT_d[:, nt * 128 : (nt + 1) * 128])
                        lhsT = lt[:]
                    else:
                        lhsT = acc[:, nt * 128 : (nt + 1) * 128]
                    pd = dpsum.tile([128, dout], f32, name="pd")
                    nc.tensor.matmul(pd[:], lhsT, W[:], start=True, stop=True)
                    hw = hwp.tile([128, dout], f32, name="hw")
                    nc.scalar.copy(hw[:], pd[:])
                    nc.sync.dma_start(agin[nt * 128 : (nt + 1) * 128, :], hw[:])

                # ---- allgather the transformed features
                nc.gpsimd.collective_compute(
                    "AllGather", mybir.AluOpType.bypass,
                    replica_groups=rg, ins=[agin.opt()], outs=[table.opt()],
                )

                # ---- aggregation per destination tile
                for t in range(ntiles):
                    gi = metap.tile([128, 4 * icols], i16, name="gi")
                    nc.sync.dma_start(gi[:], gidx_d[:, t * 4 * icols : (t + 1) * 4 * icols])
                    do = metap.tile([128, blk], f32, name="do")
                    nc.sync.dma_start(do[:], gdo_d[:, t * blk : (t + 1) * blk])
                    nm = metap.tile([128, blk], f32, name="nm")
                    nc.sync.dma_start(nm[:], gnm_d[:, t * blk : (t + 1) * blk])

                    pa = apsum.tile([dout, 128], f32, name="pa")

                    # self-loop block first (does not need the allgather)
                    sm = ltp.tile([128, dout], f32, name="sm")
                    nc.sync.dma_start(sm[:], agin[t * 128 : (t + 1) * 128, :])
                    E = ep.tile([128, 128], f32, name="E")
                    nc.vector.tensor_scalar(
                        E[:], I128[:], do[:, blk - 1 : blk], nm[:, blk - 1 : blk],
                        mybir.AluOpType.is_equal, mybir.AluOpType.mult,
                    )
                    nc.tensor.matmul(pa[:], sm[:], E[:], start=True, stop=False)

                    for w in range(4):
                        g = gthp.tile([128, bpg, dout], f32, name="g")
                        nc.gpsimd.dma_gather(
                            g[:],
                            table[w * window : (w + 1) * window, :],
                            gi[:, w * icols : (w + 1) * icols],
                            cap, cap, dout,
                        )
                        for bi in range(bpg):
                            col = w * bpg + bi
                            E = ep.tile([128, 128], f32, name="E")
                            nc.vector.tensor_scalar(
                                E[:], I128[:], do[:, col : col + 1], nm[:, col : col + 1],
                                mybir.AluOpType.is_equal, mybir.AluOpType.mult,
                            )
                            last = w == 3 and bi == bpg - 1
                            nc.tensor.matmul(
                                pa[:], g[:, bi, :], E[:], start=False, stop=last
                            )

                    # psum [dout, 128] -> +bias (+relu) -> acc / output
                    if L["relu"]:
                        nc.scalar.activation(
                            acc[:, t * 128 : (t + 1) * 128], pa[:],
                            mybir.ActivationFunctionType.Relu, bias=b[:],
                        )
                    else:
                        ot = outp.tile([d_out, 128], f32, name="ot")
                        nc.scalar.activation(
                            ot[:], pa[:],
                            mybir.ActivationFunctionType.Identity, bias=b[:],
                        )
                        nc.sync.dma_start(out_d[:, t * 128 : (t + 1) * 128], ot[:])

    nc.compile()
    return nc


# ---------------------------------------------------------------- entry point
_CACHE = {}


def _get_program():
    if "nc" not in _CACHE:
        _CACHE["nc"] = build_program(FULL_CFG)
    return _CACHE["nc"]


def kernel(**inputs):
    from concourse.bass_utils import run_bass_kernel_spmd

    cfg = FULL_CFG
    nc = _get_program()
    in_maps = prep_inputs(
        cfg,
        inputs["x"], inputs["edge_index"],
        inputs["W1"], inputs["b1"], inputs["W2"], inputs["b2"],
        inputs["W3"], inputs["b3"],
    )
    res = run_bass_kernel_spmd(nc, in_maps, list(range(N_CORES)))
    own = cfg["own"]
    parts = [res.results[c]["out"][:, :own].T for c in range(N_CORES)]
    return np.ascontiguousarray(np.concatenate(parts, axis=0), dtype=np.float32)
